# revision 5
# baseline (speedup 1.0000x reference)
"""PointSAM attention kernel for 8 Trainium2 NeuronCores.

Computation (per reference):
    scores = softplus(q @ k^T / sqrt(64) + mask * -1e9) / 2048      [B,H,L,L]
    out    = scores @ v                                             [B,H,L,D]
    return (out, scores)
(p_q / p_k are unused by the reference.)

Sharding: B*H = 32 (b,h) pairs, 4 per core (core c gets pairs 4c..4c+3,
so each core sees a single batch b = c // 4 -> one mask per core).

Device-side layout trick: everything is computed transposed.
  sT[k, q] = k @ q^T      (k on partitions -> the key mask is a per-partition
                           bias of the Exp activation: exp(s/8 - 1e9) == 0)
  softplus = Ln(Exp(s/8 + bias) + 1)   (two ACT passes; this compiler build
                                        has no softplus table set)
  outT[d, q] = sum_kb v[kb]^T @ sp[kb]  (v natural layout as stationary)
Host returns transposed views, so no on-device transposes are needed at all.
Matmuls run in float32r (~1.6e-4 rel err, 4x faster than fp32).
The 1/2048 normalization is folded into v on the host; the scores copy gets
it via the DVE eviction pass.
"""

import sys

if "/opt/trn_rl_repo" not in sys.path:
    sys.path.insert(0, "/opt/trn_rl_repo")

import numpy as np

B, H, L, D = 2, 16, 2048, 64
N_CORES = 8
PAIRS = (B * H) // N_CORES  # 4 pairs per core
KB = L // 128               # 16 k-blocks of 128
QCH = 1024                  # q chunk processed per activation instruction
QH = L // QCH               # 2 q chunks
NEG_BIG = -1.0e9

_prog_cache = {}


def _build_program():
    import concourse.bacc as bacc
    import concourse.tile as tile
    from concourse import mybir

    f32 = mybir.dt.float32
    f32r = mybir.dt.float32r
    AF = mybir.ActivationFunctionType

    nc = bacc.Bacc(
        "TRN2", target_bir_lowering=False, debug=False, num_devices=N_CORES
    )

    qT = nc.dram_tensor("qT", [PAIRS, D, L], f32, kind="ExternalInput").ap()
    kT = nc.dram_tensor("kT", [PAIRS, D, L], f32, kind="ExternalInput").ap()
    vv = nc.dram_tensor("vv", [PAIRS, 128, KB * D], f32, kind="ExternalInput").ap()
    mb = nc.dram_tensor("mb", [128, KB], f32, kind="ExternalInput").ap()
    scoresT = nc.dram_tensor(
        "scoresT", [PAIRS, L, L], f32, kind="ExternalOutput"
    ).ap()
    outT = nc.dram_tensor("outT", [PAIRS, D, L], f32, kind="ExternalOutput").ap()

    with tile.TileContext(nc) as tc:
        with (
            tc.tile_pool(name="const", bufs=1) as const_pool,
            tc.tile_pool(name="stage", bufs=2) as stage,
            tc.tile_pool(name="wpool", bufs=2) as wpool,
            tc.tile_pool(name="e_pool", bufs=3) as e_pool,
            tc.tile_pool(name="sp_pool", bufs=4) as sp_pool,
            tc.tile_pool(name="so_pool", bufs=3) as so_pool,
            tc.tile_pool(name="out_sb", bufs=2) as out_sb,
            tc.tile_pool(name="psqk", bufs=2, space="PSUM") as psum_qk,
            tc.tile_pool(name="psout", bufs=1, space="PSUM") as psum_out,
        ):
            mbt = const_pool.tile([128, KB], f32, tag="mbt")
            nc.sync.dma_start(mbt[:], mb)

            for p in range(PAIRS):
                kT32 = stage.tile([D, L], f32, tag="kT32")
                nc.sync.dma_start(kT32[:], kT[p])
                tkT = wpool.tile([D, L], f32r, tag="tkT")
                nc.vector.tensor_copy(tkT[:], kT32[:])

                qT32 = stage.tile([D, L], f32, tag="qT32")
                nc.sync.dma_start(qT32[:], qT[p])
                tqT = wpool.tile([D, L], f32r, tag="tqT")
                nc.vector.tensor_copy(tqT[:], qT32[:])

                v32 = stage.tile([128, KB * D], f32, tag="v32")
                nc.sync.dma_start(v32[:], vv[p])
                tv = wpool.tile([128, KB * D], f32r, tag="tv")
                nc.vector.tensor_copy(tv[:], v32[:])

                po = psum_out.tile([D, L], f32, tag="po")  # outT accumulator

                for kb in range(KB):
                    for qh in range(QH):
                        ps = psum_qk.tile([128, QCH], f32, tag="ps")
                        for j in range(2):
                            nc.tensor.matmul(
                                ps[:, j * 512 : (j + 1) * 512],
                                tkT[:, kb * 128 : (kb + 1) * 128],
                                tqT[:, qh * QCH + j * 512 : qh * QCH + (j + 1) * 512],
                                start=True,
                                stop=True,
                            )
                        e = e_pool.tile([128, QCH], f32, tag="e")
                        nc.scalar.activation(
                            e[:], ps[:], AF.Exp, bias=mbt[:, kb : kb + 1], scale=0.125
                        )
                        sp = sp_pool.tile([128, QCH], f32r, tag="sp")
                        nc.scalar.activation(sp[:], e[:], AF.Ln, bias=1.0)
                        so = so_pool.tile([128, QCH], f32, tag="so")
                        nc.vector.tensor_scalar_mul(so[:], sp[:], 1.0 / 2048.0)
                        nc.sync.dma_start(
                            scoresT[
                                p,
                                kb * 128 : (kb + 1) * 128,
                                qh * QCH : (qh + 1) * QCH,
                            ],
                            so[:],
                        )
                        for j in range(2):
                            nc.tensor.matmul(
                                po[:, qh * QCH + j * 512 : qh * QCH + (j + 1) * 512],
                                tv[:, kb * D : (kb + 1) * D],
                                sp[:, j * 512 : (j + 1) * 512],
                                start=(kb == 0),
                                stop=(kb == KB - 1),
                                skip_group_check=True,
                            )

                ot = out_sb.tile([D, L], f32, tag="ot")
                nc.vector.tensor_copy(ot[:], po[:])
                nc.sync.dma_start(outT[p], ot[:])

    nc.compile()
    return nc


def _get_program():
    if "nc" not in _prog_cache:
        _prog_cache["nc"] = _build_program()
    return _prog_cache["nc"]


def _prep_inputs(q, k, v, mask):
    """Build per-core input maps (host-side reshapes/transposes only)."""
    qf = np.ascontiguousarray(q, dtype=np.float32).reshape(B * H, L, D)
    kf = np.ascontiguousarray(k, dtype=np.float32).reshape(B * H, L, D)
    vf = np.ascontiguousarray(v, dtype=np.float32).reshape(B * H, L, D)
    in_maps = []
    for c in range(N_CORES):
        sl = slice(PAIRS * c, PAIRS * (c + 1))
        b = (PAIRS * c) // H
        qT_c = np.ascontiguousarray(qf[sl].transpose(0, 2, 1))  # [4, 64, 2048]
        kT_c = np.ascontiguousarray(kf[sl].transpose(0, 2, 1))  # [4, 64, 2048]
        # v: [4, 2048, 64] -> [4, 128, KB*64] with chunk kb = rows kb*128..+128
        # 1/2048 normalization folded in here (exact: power of two).
        v_c = np.ascontiguousarray(
            vf[sl].reshape(PAIRS, KB, 128, D).transpose(0, 2, 1, 3)
        ).reshape(PAIRS, 128, KB * D) * np.float32(1.0 / 2048.0)
        mb_c = np.ascontiguousarray(
            (mask[b].astype(np.float32) * np.float32(NEG_BIG)).reshape(KB, 128).T
        )
        in_maps.append({"qT": qT_c, "kT": kT_c, "vv": v_c, "mb": mb_c})
    return in_maps


def kernel(q, k, v, p_q=None, p_k=None, mask=None, **_unused):
    from concourse.bass_utils import run_bass_kernel_spmd

    assert q.shape == (B, H, L, D) and mask.shape == (B, L)
    nc = _get_program()
    in_maps = _prep_inputs(q, k, v, mask)

    res = run_bass_kernel_spmd(nc, in_maps, core_ids=list(range(N_CORES)))
    kernel.last_results = res

    scoresT_full = np.empty((B * H, L, L), dtype=np.float32)
    outT_full = np.empty((B * H, D, L), dtype=np.float32)
    for c in range(N_CORES):
        sl = slice(PAIRS * c, PAIRS * (c + 1))
        scoresT_full[sl] = res.results[c]["scoresT"]
        outT_full[sl] = res.results[c]["outT"]

    # transposed views -> full outputs (no extra 512MB copy)
    scores = scoresT_full.reshape(B, H, L, L).transpose(0, 1, 3, 2)
    out = outT_full.reshape(B, H, D, L).transpose(0, 1, 3, 2)
    return (out, scores)


def bench(q, k, v, mask, iters=16):
    """Time steady-state device execution: inputs live on device, outputs are
    never fetched. Returns estimated per-iteration time in seconds."""
    import time
    import jax
    import jax.numpy as jnp
    from jax.sharding import Mesh, PartitionSpec
    from jax.experimental.shard_map import shard_map
    from concourse import mybir
    from concourse import bass2jax
    from concourse.bass2jax import _bass_exec_p, install_neuronx_cc_hook

    install_neuronx_cc_hook()
    nc = _get_program()
    in_maps = _prep_inputs(q, k, v, mask)

    in_names, out_names, out_avals = [], [], []
    for alloc in nc.m.functions[0].allocations:
        if not isinstance(alloc, mybir.MemoryLocationSet):
            continue
        name = alloc.memorylocations[0].name
        if alloc.kind == "ExternalInput":
            in_names.append(name)
        elif alloc.kind == "ExternalOutput":
            out_names.append(name)
            out_avals.append(
                jax.core.ShapedArray(
                    tuple(alloc.tensor_shape), mybir.dt.np(alloc.dtype)
                )
            )
    partition_name = (
        nc.partition_id_tensor.name if nc.partition_id_tensor else None
    )
    if partition_name is not None:
        in_names = [n for n in in_names if n != partition_name]
    n_params = len(in_names)
    all_in_names = in_names + out_names
    if partition_name is not None:
        all_in_names = all_in_names + [partition_name]

    def _body(*args):
        operands = list(args)
        if partition_name is not None:
            operands.append(bass2jax.partition_id_tensor())
        outs = _bass_exec_p.bind(
            *operands,
            out_avals=tuple(out_avals),
            in_names=tuple(all_in_names),
            out_names=tuple(out_names),
            lowering_input_output_aliases=(),
            sim_require_finite=True,
            sim_require_nnan=True,
            nc=nc,
        )
        return tuple(outs)

    devices = jax.devices()[:N_CORES]
    mesh = Mesh(np.asarray(devices), ("core",))
    nspec = n_params + len(out_names)
    f = jax.jit(
        shard_map(
            _body,
            mesh=mesh,
            in_specs=(PartitionSpec("core"),) * nspec,
            out_specs=(PartitionSpec("core"),) * len(out_names),
            check_rep=False,
        ),
        keep_unused=True,
    )
    sh = jax.sharding.NamedSharding(mesh, PartitionSpec("core"))
    concat_in = [
        jax.device_put(
            np.concatenate([in_maps[c][nm] for c in range(N_CORES)], axis=0), sh
        )
        for nm in in_names
    ]
    zeros = [
        jax.device_put(
            np.zeros((N_CORES * av.shape[0], *av.shape[1:]), av.dtype), sh
        )
        for av in out_avals
    ]
    args = concat_in + zeros

    outs = f(*args)  # compile + warmup
    jax.block_until_ready(outs)

    def timed(n):
        t0 = time.perf_counter()
        o = None
        for _ in range(n):
            o = f(*args)
        jax.block_until_ready(o)
        return time.perf_counter() - t0

    t1 = timed(2)
    tN = timed(iters)
    per_iter = (tN - t1) / (iters - 2)
    return per_iter, t1 / 2, tN / iters


# revision 7
# speedup vs baseline: 2.6874x; 2.6874x over previous
"""PointSAM attention kernel for 8 Trainium2 NeuronCores.

Computation (per reference):
    scores = softplus(q @ k^T / sqrt(64) + mask * -1e9) / 2048      [B,H,L,L]
    out    = scores @ v                                             [B,H,L,D]
    return (out, scores)
(p_q / p_k are unused by the reference.)

Sharding: B*H = 32 (b,h) pairs, 4 per core (core c gets pairs 4c..4c+3,
so each core sees a single batch b = c // 4 -> one mask per core).

Device-side layout trick: everything is computed transposed.
  sT[k, q] = k @ q^T      (k on partitions -> the key mask is a per-partition
                           bias of the Exp activation: exp(s/8 - 1e9) == 0)
  softplus = Ln(Exp(s/8 + bias) + 1)   (two ACT passes; this compiler build
                                        has no softplus table set)
  outT[d, q] = sum_kb v[kb]^T @ sp[kb]  (v natural layout as stationary)
Host returns transposed views, so no on-device transposes are needed at all.
Matmuls run in float32r (~1.6e-4 rel err, 4x faster than fp32).
The 1/2048 normalization is folded into v on the host; the scores copy gets
it via the DVE eviction pass.
"""

import sys

if "/opt/trn_rl_repo" not in sys.path:
    sys.path.insert(0, "/opt/trn_rl_repo")

import numpy as np

B, H, L, D = 2, 16, 2048, 64
N_CORES = 8
PAIRS = (B * H) // N_CORES  # 4 pairs per core
KB = L // 128               # 16 k-blocks of 128
QCH = 1024                  # q chunk processed per activation instruction
QH = L // QCH               # 2 q chunks
NEG_BIG = -1.0e9

_prog_cache = {}


def _patch_act_tables():
    """Make natural_log_exp_and_others the only set advertising Exp/Ln so the
    table-load pass keeps one table resident instead of thrashing between
    exp_and_others and natural_log on every Exp/Ln alternation (~2.7us per
    reload). Entry order (= act_func_set_id) is preserved."""
    import concourse.bacc as bacc
    import concourse.hw_specs as hw_specs
    from concourse import mybir

    if getattr(_patch_act_tables, "_done", False):
        return
    orig = hw_specs.get_activation_tables

    def patched(arch):
        tabs = orig(arch)
        strip = {mybir.ActivationFunctionType.Exp, mybir.ActivationFunctionType.Ln}
        for name in tabs:
            if name != "natural_log_exp_and_others":
                tabs[name] = set(tabs[name]) - strip
        return tabs

    hw_specs.get_activation_tables = patched
    bacc.get_activation_tables = patched
    _patch_act_tables._done = True


def _build_program():
    import concourse.bacc as bacc
    import concourse.tile as tile
    from concourse import mybir

    _patch_act_tables()

    f32 = mybir.dt.float32
    f32r = mybir.dt.float32r
    AF = mybir.ActivationFunctionType

    nc = bacc.Bacc(
        "TRN2", target_bir_lowering=False, debug=False, num_devices=N_CORES
    )

    qT = nc.dram_tensor("qT", [PAIRS, D, L], f32, kind="ExternalInput").ap()
    kT = nc.dram_tensor("kT", [PAIRS, D, L], f32, kind="ExternalInput").ap()
    vv = nc.dram_tensor("vv", [PAIRS, 128, KB * D], f32, kind="ExternalInput").ap()
    mb = nc.dram_tensor("mb", [128, KB], f32, kind="ExternalInput").ap()
    scoresT = nc.dram_tensor(
        "scoresT", [PAIRS, L, L], f32, kind="ExternalOutput"
    ).ap()
    outT = nc.dram_tensor("outT", [PAIRS, D, L], f32, kind="ExternalOutput").ap()

    with tile.TileContext(nc) as tc:
        with (
            tc.tile_pool(name="const", bufs=1) as const_pool,
            tc.tile_pool(name="stage", bufs=2) as stage,
            tc.tile_pool(name="wpool", bufs=2) as wpool,
            tc.tile_pool(name="e_pool", bufs=3) as e_pool,
            tc.tile_pool(name="sp_pool", bufs=4) as sp_pool,
            tc.tile_pool(name="so_pool", bufs=3) as so_pool,
            tc.tile_pool(name="out_sb", bufs=2) as out_sb,
            tc.tile_pool(name="psqk", bufs=2, space="PSUM") as psum_qk,
            tc.tile_pool(name="psout", bufs=1, space="PSUM") as psum_out,
        ):
            mbt = const_pool.tile([128, KB], f32, tag="mbt")
            nc.sync.dma_start(mbt[:], mb)

            for p in range(PAIRS):
                kT32 = stage.tile([D, L], f32, tag="kT32")
                nc.sync.dma_start(kT32[:], kT[p])
                tkT = wpool.tile([D, L], f32r, tag="tkT")
                nc.vector.tensor_copy(tkT[:], kT32[:])

                qT32 = stage.tile([D, L], f32, tag="qT32")
                nc.sync.dma_start(qT32[:], qT[p])
                tqT = wpool.tile([D, L], f32r, tag="tqT")
                nc.vector.tensor_copy(tqT[:], qT32[:])

                v32 = stage.tile([128, KB * D], f32, tag="v32")
                nc.sync.dma_start(v32[:], vv[p])
                tv = wpool.tile([128, KB * D], f32r, tag="tv")
                nc.vector.tensor_copy(tv[:], v32[:])

                po = psum_out.tile([D, L], f32, tag="po")  # outT accumulator

                for kb in range(KB):
                    e = e_pool.tile([128, L], f32, tag="e")
                    for qh in range(QH):
                        ps = psum_qk.tile([128, QCH], f32, tag="ps")
                        for j in range(2):
                            nc.tensor.matmul(
                                ps[:, j * 512 : (j + 1) * 512],
                                tkT[:, kb * 128 : (kb + 1) * 128],
                                tqT[:, qh * QCH + j * 512 : qh * QCH + (j + 1) * 512],
                                start=True,
                                stop=True,
                            )
                        nc.scalar.activation(
                            e[:, qh * QCH : (qh + 1) * QCH],
                            ps[:],
                            AF.Exp,
                            bias=mbt[:, kb : kb + 1],
                            scale=0.125,
                        )
                    sp = sp_pool.tile([128, L], f32r, tag="sp")
                    nc.scalar.activation(sp[:], e[:], AF.Ln, bias=1.0)
                    so = so_pool.tile([128, L], f32, tag="so")
                    nc.vector.tensor_scalar_mul(so[:], sp[:], 1.0 / 2048.0)
                    nc.sync.dma_start(
                        scoresT[p, kb * 128 : (kb + 1) * 128, :], so[:]
                    )
                    for j in range(4):
                        nc.tensor.matmul(
                            po[:, j * 512 : (j + 1) * 512],
                            tv[:, kb * D : (kb + 1) * D],
                            sp[:, j * 512 : (j + 1) * 512],
                            start=(kb == 0),
                            stop=(kb == KB - 1),
                            skip_group_check=True,
                        )

                ot = out_sb.tile([D, L], f32, tag="ot")
                nc.vector.tensor_copy(ot[:], po[:])
                nc.sync.dma_start(outT[p], ot[:])

    nc.compile()
    return nc


def _get_program():
    if "nc" not in _prog_cache:
        _prog_cache["nc"] = _build_program()
    return _prog_cache["nc"]


def _prep_inputs(q, k, v, mask):
    """Build per-core input maps (host-side reshapes/transposes only)."""
    qf = np.ascontiguousarray(q, dtype=np.float32).reshape(B * H, L, D)
    kf = np.ascontiguousarray(k, dtype=np.float32).reshape(B * H, L, D)
    vf = np.ascontiguousarray(v, dtype=np.float32).reshape(B * H, L, D)
    in_maps = []
    for c in range(N_CORES):
        sl = slice(PAIRS * c, PAIRS * (c + 1))
        b = (PAIRS * c) // H
        qT_c = np.ascontiguousarray(qf[sl].transpose(0, 2, 1))  # [4, 64, 2048]
        kT_c = np.ascontiguousarray(kf[sl].transpose(0, 2, 1))  # [4, 64, 2048]
        # v: [4, 2048, 64] -> [4, 128, KB*64] with chunk kb = rows kb*128..+128
        # 1/2048 normalization folded in here (exact: power of two).
        v_c = np.ascontiguousarray(
            vf[sl].reshape(PAIRS, KB, 128, D).transpose(0, 2, 1, 3)
        ).reshape(PAIRS, 128, KB * D) * np.float32(1.0 / 2048.0)
        mb_c = np.ascontiguousarray(
            (mask[b].astype(np.float32) * np.float32(NEG_BIG)).reshape(KB, 128).T
        )
        in_maps.append({"qT": qT_c, "kT": kT_c, "vv": v_c, "mb": mb_c})
    return in_maps


def kernel(q, k, v, p_q=None, p_k=None, mask=None, **_unused):
    from concourse.bass_utils import run_bass_kernel_spmd

    assert q.shape == (B, H, L, D) and mask.shape == (B, L)
    nc = _get_program()
    in_maps = _prep_inputs(q, k, v, mask)

    res = run_bass_kernel_spmd(nc, in_maps, core_ids=list(range(N_CORES)))
    kernel.last_results = res

    scoresT_full = np.empty((B * H, L, L), dtype=np.float32)
    outT_full = np.empty((B * H, D, L), dtype=np.float32)
    for c in range(N_CORES):
        sl = slice(PAIRS * c, PAIRS * (c + 1))
        scoresT_full[sl] = res.results[c]["scoresT"]
        outT_full[sl] = res.results[c]["outT"]

    # transposed views -> full outputs (no extra 512MB copy)
    scores = scoresT_full.reshape(B, H, L, L).transpose(0, 1, 3, 2)
    out = outT_full.reshape(B, H, D, L).transpose(0, 1, 3, 2)
    return (out, scores)


def bench(q, k, v, mask, iters=16):
    """Time steady-state device execution: inputs live on device, outputs are
    never fetched. Returns estimated per-iteration time in seconds."""
    import time
    import jax
    import jax.numpy as jnp
    from jax.sharding import Mesh, PartitionSpec
    from jax.experimental.shard_map import shard_map
    from concourse import mybir
    from concourse import bass2jax
    from concourse.bass2jax import _bass_exec_p, install_neuronx_cc_hook

    install_neuronx_cc_hook()
    nc = _get_program()
    in_maps = _prep_inputs(q, k, v, mask)

    in_names, out_names, out_avals = [], [], []
    for alloc in nc.m.functions[0].allocations:
        if not isinstance(alloc, mybir.MemoryLocationSet):
            continue
        name = alloc.memorylocations[0].name
        if alloc.kind == "ExternalInput":
            in_names.append(name)
        elif alloc.kind == "ExternalOutput":
            out_names.append(name)
            out_avals.append(
                jax.core.ShapedArray(
                    tuple(alloc.tensor_shape), mybir.dt.np(alloc.dtype)
                )
            )
    partition_name = (
        nc.partition_id_tensor.name if nc.partition_id_tensor else None
    )
    if partition_name is not None:
        in_names = [n for n in in_names if n != partition_name]
    n_params = len(in_names)
    all_in_names = in_names + out_names
    if partition_name is not None:
        all_in_names = all_in_names + [partition_name]

    def _body(*args):
        operands = list(args)
        if partition_name is not None:
            operands.append(bass2jax.partition_id_tensor())
        outs = _bass_exec_p.bind(
            *operands,
            out_avals=tuple(out_avals),
            in_names=tuple(all_in_names),
            out_names=tuple(out_names),
            lowering_input_output_aliases=(),
            sim_require_finite=True,
            sim_require_nnan=True,
            nc=nc,
        )
        return tuple(outs)

    devices = jax.devices()[:N_CORES]
    mesh = Mesh(np.asarray(devices), ("core",))
    nspec = n_params + len(out_names)
    f = jax.jit(
        shard_map(
            _body,
            mesh=mesh,
            in_specs=(PartitionSpec("core"),) * nspec,
            out_specs=(PartitionSpec("core"),) * len(out_names),
            check_rep=False,
        ),
        keep_unused=True,
    )
    sh = jax.sharding.NamedSharding(mesh, PartitionSpec("core"))
    concat_in = [
        jax.device_put(
            np.concatenate([in_maps[c][nm] for c in range(N_CORES)], axis=0), sh
        )
        for nm in in_names
    ]
    zeros = [
        jax.device_put(
            np.zeros((N_CORES * av.shape[0], *av.shape[1:]), av.dtype), sh
        )
        for av in out_avals
    ]
    args = concat_in + zeros

    outs = f(*args)  # compile + warmup
    jax.block_until_ready(outs)

    def timed(n):
        t0 = time.perf_counter()
        o = None
        for _ in range(n):
            o = f(*args)
        jax.block_until_ready(o)
        return time.perf_counter() - t0

    t1 = timed(2)
    tN = timed(iters)
    per_iter = (tN - t1) / (iters - 2)
    return per_iter, t1 / 2, tN / iters


# revision 11
# speedup vs baseline: 7.3781x; 2.7454x over previous
"""PointSAM attention kernel for 8 Trainium2 NeuronCores.

Computation (per reference):
    scores = softplus(q @ k^T / sqrt(64) + mask * -1e9) / 2048      [B,H,L,L]
    out    = scores @ v                                             [B,H,L,D]
    return (out, scores)
(p_q / p_k are unused by the reference.)

Sharding: B*H = 32 (b,h) pairs, 4 per core (core c gets pairs 4c..4c+3,
so each core sees a single batch b = c // 4 -> one mask per core).

Key tricks:
- Masked keys produce EXACTLY 0 in both outputs (softplus(-1e9)=0), so the
  k-dimension is compacted on the host to the unmasked keys only (~half):
  the device computes a [NU, L] score block per pair; the host scatters rows
  into a zeros buffer. Halves ACT/PE/DVE work and the dominant scores DMA.
- Everything is computed transposed: sT[k,q] = kT.T @ qT (k on partitions),
  outT[d,q] accumulated with v-chunks as the stationary operand. The host
  returns transposed views, so there are no on-device transposes at all.
- softplus = Ln(Exp(x)+1): this compiler build has no softplus act table.
  Both Exp and Ln live in the natural_log_exp_and_others set; a patch to
  get_activation_tables stops the table-load pass from thrashing sets.
- Matmuls run in float32r (~1.6e-4 rel err, bf16-class speed, 4x over fp32).
- The 1/2048 normalization is folded into v on the host; the scores copy
  gets it in the DVE eviction pass.
"""

import sys

if "/opt/trn_rl_repo" not in sys.path:
    sys.path.insert(0, "/opt/trn_rl_repo")

import numpy as np

B, H, L, D = 2, 16, 2048, 64
N_CORES = 8
PAIRS = (B * H) // N_CORES  # 4 pairs per core
QCH = 1024                  # q chunk per Exp instruction (psum tile width)
QH = L // QCH
NEG_BIG = -1.0e9

_prog_cache = {}


def _patch_act_tables():
    """Make natural_log_exp_and_others the only set advertising Exp/Ln so the
    table-load pass keeps one table resident instead of reloading on every
    Exp/Ln alternation (~2.7us per reload). Entry order (= act_func_set_id)
    is preserved."""
    import concourse.bacc as bacc
    import concourse.hw_specs as hw_specs
    from concourse import mybir

    if getattr(_patch_act_tables, "_done", False):
        return
    orig = hw_specs.get_activation_tables

    def patched(arch):
        tabs = orig(arch)
        strip = {mybir.ActivationFunctionType.Exp, mybir.ActivationFunctionType.Ln}
        for name in tabs:
            if name != "natural_log_exp_and_others":
                tabs[name] = set(tabs[name]) - strip
        return tabs

    hw_specs.get_activation_tables = patched
    bacc.get_activation_tables = patched
    _patch_act_tables._done = True


def _build_program(nu, repeat=1):
    """Build the SPMD program for NU (padded, multiple of 128) unmasked keys."""
    import concourse.bacc as bacc
    import concourse.tile as tile
    from concourse import mybir

    _patch_act_tables()

    f32 = mybir.dt.float32
    f32r = mybir.dt.float32r
    AF = mybir.ActivationFunctionType
    KBk = nu // 128

    nc = bacc.Bacc(
        "TRN2", target_bir_lowering=False, debug=False, num_devices=N_CORES
    )

    qT = nc.dram_tensor("qT", [PAIRS, D, L], f32, kind="ExternalInput").ap()
    kT = nc.dram_tensor("kT", [PAIRS, D, nu], f32, kind="ExternalInput").ap()
    vv = nc.dram_tensor("vv", [PAIRS, 128, KBk * D], f32, kind="ExternalInput").ap()
    scoresT = nc.dram_tensor(
        "scoresT", [PAIRS, nu, L], f32, kind="ExternalOutput"
    ).ap()
    outT = nc.dram_tensor("outT", [PAIRS, D, L], f32, kind="ExternalOutput").ap()

    with tile.TileContext(nc) as tc:
        with (
            tc.tile_pool(name="stage", bufs=2) as stage,
            tc.tile_pool(name="wpool", bufs=2) as wpool,
            tc.tile_pool(name="e_pool", bufs=3) as e_pool,
            tc.tile_pool(name="sp_pool", bufs=4) as sp_pool,
            tc.tile_pool(name="so_pool", bufs=3) as so_pool,
            tc.tile_pool(name="out_sb", bufs=2) as out_sb,
            tc.tile_pool(name="psqk", bufs=2, space="PSUM") as psum_qk,
            tc.tile_pool(name="psout", bufs=1, space="PSUM") as psum_out,
        ):
            for p in list(range(PAIRS)) * repeat:
                kT32 = stage.tile([D, nu], f32, tag="kT32")
                nc.sync.dma_start(kT32[:], kT[p])
                tkT = wpool.tile([D, nu], f32r, tag="tkT")
                nc.vector.tensor_copy(tkT[:], kT32[:])

                qT32 = stage.tile([D, L], f32, tag="qT32")
                nc.sync.dma_start(qT32[:], qT[p])
                tqT = wpool.tile([D, L], f32r, tag="tqT")
                nc.vector.tensor_copy(tqT[:], qT32[:])

                v32 = stage.tile([128, KBk * D], f32, tag="v32")
                nc.sync.dma_start(v32[:], vv[p])
                tv = wpool.tile([128, KBk * D], f32r, tag="tv")
                nc.vector.tensor_copy(tv[:], v32[:])

                po = psum_out.tile([D, L], f32, tag="po")  # outT accumulator

                for kb in range(KBk):
                    e = e_pool.tile([128, L], f32, tag="e")
                    for qh in range(QH):
                        ps = psum_qk.tile([128, QCH], f32, tag="ps")
                        for j in range(2):
                            nc.tensor.matmul(
                                ps[:, j * 512 : (j + 1) * 512],
                                tkT[:, kb * 128 : (kb + 1) * 128],
                                tqT[:, qh * QCH + j * 512 : qh * QCH + (j + 1) * 512],
                                start=True,
                                stop=True,
                            )
                        nc.scalar.activation(
                            e[:, qh * QCH : (qh + 1) * QCH],
                            ps[:],
                            AF.Exp,
                            scale=0.125,
                        )
                    sp = sp_pool.tile([128, L], f32r, tag="sp")
                    nc.scalar.activation(sp[:], e[:], AF.Ln, bias=1.0)
                    so = so_pool.tile([128, L], f32, tag="so")
                    nc.vector.tensor_scalar_mul(so[:], sp[:], 1.0 / 2048.0)
                    nc.sync.dma_start(
                        scoresT[p, kb * 128 : (kb + 1) * 128, :], so[:]
                    )
                    for j in range(4):
                        nc.tensor.matmul(
                            po[:, j * 512 : (j + 1) * 512],
                            tv[:, kb * D : (kb + 1) * D],
                            sp[:, j * 512 : (j + 1) * 512],
                            start=(kb == 0),
                            stop=(kb == KBk - 1),
                            skip_group_check=True,
                        )

                ot = out_sb.tile([D, L], f32, tag="ot")
                nc.vector.tensor_copy(ot[:], po[:])
                nc.sync.dma_start(outT[p], ot[:])

    nc.compile()
    return nc


def _get_program(nu, repeat=1):
    key = (nu, repeat)
    if key not in _prog_cache:
        _prog_cache[key] = _build_program(nu, repeat)
    return _prog_cache[key]


def _prep(q, k, v, mask):
    """Host-side compaction + per-core input maps. Returns (nu_pad, ku per
    batch, in_maps)."""
    qf = np.ascontiguousarray(q, dtype=np.float32).reshape(B * H, L, D)
    kf = np.ascontiguousarray(k, dtype=np.float32).reshape(B * H, L, D)
    vf = np.ascontiguousarray(v, dtype=np.float32).reshape(B * H, L, D)
    mask = np.asarray(mask)

    ku = [np.where(mask[b] == 0)[0] for b in range(B)]
    nu_max = max(1, max(len(x) for x in ku))
    nu_pad = ((nu_max + 127) // 128) * 128
    KBk = nu_pad // 128

    in_maps = []
    for c in range(N_CORES):
        sl = slice(PAIRS * c, PAIRS * (c + 1))
        b = (PAIRS * c) // H
        kidx = ku[b]
        nvalid = len(kidx)
        # pad with key 0 (v rows for padding are zeroed -> no contribution)
        kidx_pad = np.zeros(nu_pad, dtype=np.int64)
        kidx_pad[:nvalid] = kidx

        qT_c = np.ascontiguousarray(qf[sl].transpose(0, 2, 1))  # [4, 64, L]
        kg = kf[sl][:, kidx_pad, :]                             # [4, nu_pad, 64]
        kT_c = np.ascontiguousarray(kg.transpose(0, 2, 1))      # [4, 64, nu_pad]
        vg = vf[sl][:, kidx_pad, :] * np.float32(1.0 / 2048.0)  # [4, nu_pad, 64]
        vg[:, nvalid:, :] = 0.0
        v_c = np.ascontiguousarray(
            vg.reshape(PAIRS, KBk, 128, D).transpose(0, 2, 1, 3)
        ).reshape(PAIRS, 128, KBk * D)
        in_maps.append({"qT": qT_c, "kT": kT_c, "vv": v_c})
    return nu_pad, ku, in_maps


def kernel(q, k, v, p_q=None, p_k=None, mask=None, **_unused):
    from concourse.bass_utils import run_bass_kernel_spmd

    q = np.asarray(q)
    mask = np.asarray(mask)
    assert q.shape == (B, H, L, D) and mask.shape == (B, L)

    nu_pad, ku, in_maps = _prep(q, k, v, mask)
    nc = _get_program(nu_pad)

    res = run_bass_kernel_spmd(nc, in_maps, core_ids=list(range(N_CORES)))
    kernel.last_results = res

    scoresT_full = np.zeros((B * H, L, L), dtype=np.float32)
    outT_full = np.empty((B * H, D, L), dtype=np.float32)
    for c in range(N_CORES):
        b = (PAIRS * c) // H
        kidx = ku[b]
        nvalid = len(kidx)
        sc = res.results[c]["scoresT"]  # [PAIRS, nu_pad, L]
        for i in range(PAIRS):
            scoresT_full[PAIRS * c + i][kidx] = sc[i, :nvalid]
        outT_full[PAIRS * c : PAIRS * (c + 1)] = res.results[c]["outT"]

    # transposed views -> full outputs (no extra 512MB copy)
    scores = scoresT_full.reshape(B, H, L, L).transpose(0, 1, 3, 2)
    out = outT_full.reshape(B, H, D, L).transpose(0, 1, 3, 2)
    return (out, scores)


def bench(q, k, v, mask, iters=16):
    """Time steady-state device execution: inputs live on device, outputs are
    never fetched. Returns estimated per-iteration time in seconds."""
    import time
    import jax
    from jax.sharding import Mesh, PartitionSpec, NamedSharding
    from jax.experimental.shard_map import shard_map
    from concourse import mybir
    from concourse import bass2jax
    from concourse.bass2jax import _bass_exec_p, install_neuronx_cc_hook

    install_neuronx_cc_hook()
    nu_pad, ku, in_maps = _prep(q, k, v, np.asarray(mask))
    nc = _get_program(nu_pad)

    in_names, out_names, out_avals = [], [], []
    for alloc in nc.m.functions[0].allocations:
        if not isinstance(alloc, mybir.MemoryLocationSet):
            continue
        name = alloc.memorylocations[0].name
        if alloc.kind == "ExternalInput":
            in_names.append(name)
        elif alloc.kind == "ExternalOutput":
            out_names.append(name)
            out_avals.append(
                jax.core.ShapedArray(
                    tuple(alloc.tensor_shape), mybir.dt.np(alloc.dtype)
                )
            )
    partition_name = (
        nc.partition_id_tensor.name if nc.partition_id_tensor else None
    )
    if partition_name is not None:
        in_names = [n for n in in_names if n != partition_name]
    all_in_names = in_names + out_names
    if partition_name is not None:
        all_in_names = all_in_names + [partition_name]

    def _body(*args):
        operands = list(args)
        if partition_name is not None:
            operands.append(bass2jax.partition_id_tensor())
        outs = _bass_exec_p.bind(
            *operands,
            out_avals=tuple(out_avals),
            in_names=tuple(all_in_names),
            out_names=tuple(out_names),
            lowering_input_output_aliases=(),
            sim_require_finite=True,
            sim_require_nnan=True,
            nc=nc,
        )
        return tuple(outs)

    devices = jax.devices()[:N_CORES]
    mesh = Mesh(np.asarray(devices), ("core",))
    nspec = len(in_names) + len(out_names)
    f = jax.jit(
        shard_map(
            _body,
            mesh=mesh,
            in_specs=(PartitionSpec("core"),) * nspec,
            out_specs=(PartitionSpec("core"),) * len(out_names),
            check_rep=False,
        ),
        keep_unused=True,
    )
    sh = NamedSharding(mesh, PartitionSpec("core"))
    concat_in = [
        jax.device_put(
            np.concatenate([in_maps[c][nm] for c in range(N_CORES)], axis=0), sh
        )
        for nm in in_names
    ]
    zeros = tuple(
        jax.device_put(
            np.zeros((N_CORES * av.shape[0], *av.shape[1:]), av.dtype), sh
        )
        for av in out_avals
    )
    iout = out_names.index("outT")

    outs = f(*concat_in, *zeros)
    _ = np.asarray(outs[iout])

    def run(n):
        o = outs
        t0 = time.perf_counter()
        for _ in range(n):
            o = f(*concat_in, *o)
        _ = np.asarray(o[iout])
        return time.perf_counter() - t0

    slopes = []
    for _ in range(4):
        t2 = run(2)
        tN = run(iters)
        slopes.append((tN - t2) / (iters - 2))
    med = float(np.median(slopes))
    return med, min(slopes), slopes
